# revision 1
# baseline (speedup 1.0000x reference)
"""Trainium2 Bass kernel for the DependencyParseModel problem.

Pipeline (replicated biLSTM, pairwise scoring sharded over 8 cores):
  1. host: embedding gather, weight permute/pad into PE-friendly layouts
  2. device: XW = Wih @ x_aug  (dense matmuls), 256-step LSTM scans with
     weights-stationary matvecs (bf16 FWL), both directions interleaved
  3. device: pairwise-MLP scoring for this core's 32 head rows
  4. host: assemble [256,256], add b2, zero diagonal

Self-contained: hardcodes all shapes; no sibling imports.
"""

import os
import numpy as np
import ml_dtypes

T = 256
H = 400            # LSTM hidden
HP = 512           # padded hidden
G = 2048           # padded gate dim (4 gates x 512)
D1P = 1024         # padded layer-1 input dim (2 x HP)
MLP = 400
NCORES = 8
RPC = T // NCORES  # rows per core (head-word rows)

BF16 = ml_dtypes.bfloat16

# gate blocks in OUR layout order [i, f, o, g~]; source ranges in torch order
_GATE_SRC = [(0, 400), (400, 800), (1200, 1600), (800, 1200)]


def _permute_pad_gate_rows(W):
    """[1600, K] -> [2048, K]: torch gate order i,f,g,o -> blocks [i,f,o,g~], each padded to 512."""
    out = np.zeros((G, W.shape[1]), np.float32)
    for b, (s, e) in enumerate(_GATE_SRC):
        out[b * 512: b * 512 + (e - s)] = W[s:e]
    return out


def _permute_pad_gate_vec(v):
    out = np.zeros(G, np.float32)
    for b, (s, e) in enumerate(_GATE_SRC):
        out[b * 512: b * 512 + (e - s)] = v[s:e]
    return out


def _pad_cols(W, K):
    """[R, k] -> [R, K] zero-padded."""
    out = np.zeros((W.shape[0], K), np.float32)
    out[:, : W.shape[1]] = W
    return out


def _h_tile(v):
    """[400] -> [128, 4] tile, elem d -> (d % 128, d // 128)."""
    out = np.zeros(HP, np.float32)
    out[:H] = v
    return np.ascontiguousarray(out.reshape(4, 128).T)


_PROG_CACHE = {}


def _get_program(n_steps=T):
    key = ("prog", n_steps)
    if key in _PROG_CACHE:
        return _PROG_CACHE[key]

    import concourse.bass as bass
    import concourse.mybir as mybir
    import concourse.tile as tile
    from concourse import bacc

    F32 = mybir.dt.float32
    BF = mybir.dt.bfloat16
    AF = mybir.ActivationFunctionType
    ALU = mybir.AluOpType

    nc = bacc.Bacc("TRN2", target_bir_lowering=False, debug=False,
                   enable_asserts=False, num_devices=NCORES)

    # ---- DRAM I/O ----
    d_xT = nc.dram_tensor("xT", [HP, T], BF, kind="ExternalInput").ap()
    d_wihT0 = [nc.dram_tensor(f"wihT0{d}", [HP, G], BF, kind="ExternalInput").ap() for d in "fb"]
    d_wihT1 = [nc.dram_tensor(f"wihT1{d}", [D1P, G], BF, kind="ExternalInput").ap() for d in "fb"]
    d_whhT = [[nc.dram_tensor(f"whhT{l}{d}", [HP, G], BF, kind="ExternalInput").ap() for d in "fb"]
              for l in (0, 1)]
    d_h0 = [[nc.dram_tensor(f"h0_{l}{d}", [128, 4], BF, kind="ExternalInput").ap() for d in "fb"]
            for l in (0, 1)]
    d_c0 = [[nc.dram_tensor(f"c0_{l}{d}", [128, 4], F32, kind="ExternalInput").ap() for d in "fb"]
            for l in (0, 1)]
    d_w1aT = nc.dram_tensor("w1aT", [D1P, MLP], BF, kind="ExternalInput").ap()
    d_w1bT = nc.dram_tensor("w1bT", [D1P, MLP], BF, kind="ExternalInput").ap()
    d_skT = nc.dram_tensor("skT", [T, RPC], BF, kind="ExternalInput").ap()
    d_w2rep = nc.dram_tensor("w2rep", [128, MLP], BF, kind="ExternalInput").ap()
    d_ident = nc.dram_tensor("ident", [128, 128], BF, kind="ExternalInput").ap()
    d_out = nc.dram_tensor("scores_t", [T, RPC], F32, kind="ExternalOutput").ap()

    with tile.TileContext(nc) as tc:
        from contextlib import ExitStack
        with ExitStack() as ctx:
            const = ctx.enter_context(tc.tile_pool(name="const", bufs=1))
            state = ctx.enter_context(tc.tile_pool(name="state", bufs=1))
            whhp = ctx.enter_context(tc.tile_pool(name="whhp", bufs=1))

            # --- constants / initial state in SBUF ---
            xT_sb = []
            for kc in range(4):
                xt = const.tile([128, T], BF, name=f"xT{kc}")
                nc.sync.dma_start(xt, d_xT[128 * kc:128 * (kc + 1), :])
                xT_sb.append(xt)
            h0_sb = [[None, None], [None, None]]
            c_sb = [[None, None], [None, None]]
            for l in (0, 1):
                for d in (0, 1):
                    t0 = const.tile([128, 4], BF, name=f"h0sb{l}{d}")
                    nc.sync.dma_start(t0, d_h0[l][d])
                    h0_sb[l][d] = t0
                    t1 = state.tile([128, 4], F32, name=f"csb{l}{d}")
                    nc.sync.dma_start(t1, d_c0[l][d])
                    c_sb[l][d] = t1
            whh_sb = [[None, None], [None, None]]
            for l in (0, 1):
                for d in (0, 1):
                    chunks = []
                    for kc in range(4):
                        w = whhp.tile([128, G], BF, name=f"whh{l}{d}{kc}")
                        nc.sync.dma_start(w, d_whhT[l][d][128 * kc:128 * (kc + 1), :])
                        chunks.append(w)
                    whh_sb[l][d] = chunks
            w1aT_sb, w1bT_sb = [], []
            for kc in range(8):
                wa = const.tile([128, MLP], BF, name=f"w1aT{kc}")
                nc.sync.dma_start(wa, d_w1aT[128 * kc:128 * (kc + 1), :])
                w1aT_sb.append(wa)
                wb = const.tile([128, MLP], BF, name=f"w1bT{kc}")
                nc.sync.dma_start(wb, d_w1bT[128 * kc:128 * (kc + 1), :])
                w1bT_sb.append(wb)
            skT_sb = []
            for kc in range(2):
                sk = const.tile([128, RPC], BF, name=f"skT{kc}")
                nc.sync.dma_start(sk, d_skT[128 * kc:128 * (kc + 1), :])
                skT_sb.append(sk)
            w2rep_sb = const.tile([128, MLP], BF, name="w2rep")
            nc.sync.dma_start(w2rep_sb, d_w2rep)
            ident_sb = const.tile([128, 128], BF, name="ident")
            nc.sync.dma_start(ident_sb, d_ident)
            ones_sb = const.tile([1, 128], BF, name="ones")
            nc.vector.memset(ones_sb, 1.0)

            # persistent per-(layer,dir) h history [128, 4*T] bf16, col 4t+c = h_t[128c+p]
            hh_sb = [[state.tile([128, 4 * T], BF, name=f"hh{l}{d}") for d in (0, 1)]
                     for l in (0, 1)]
            # XW^T buffers, reused across layers: [128, 16*T] bf16, col m*T + t
            # (bf16 so the per-step XW add can ride the PE as an identity-matmul)
            xwt_sb = [state.tile([128, 16 * T], BF, name=f"xwt{d}") for d in (0, 1)]

            def xwt_phase(layer, preloaded=None):
                """xwt_sb[d] <- Wih[layer][d] @ x_aug (all timesteps)."""
                K = 4 if layer == 0 else 8
                d_wih = d_wihT0 if layer == 0 else d_wihT1
                with tc.tile_pool(name=f"wihp{layer}", bufs=1) as wp, \
                     tc.tile_pool(name=f"xwps{layer}", bufs=4, space="PSUM") as pp:
                    for d in (0, 1):
                        if preloaded is not None:
                            wih_sb = preloaded[d]
                        else:
                            wih_sb = []
                            for kc in range(K):
                                w = wp.tile([128, G], BF, name=f"wih{layer}{d}{kc}",
                                            tag=f"wih{kc}")
                                nc.sync.dma_start(w, d_wih[d][128 * kc:128 * (kc + 1), :])
                                wih_sb.append(w)
                        if layer == 0:
                            rhs = xT_sb
                        else:
                            rhs = []
                            for kc in range(K):
                                hhr = hh_sb[0][kc // 4][:].rearrange(
                                    "p (t c) -> p c t", c=4)
                                rhs.append(hhr[:, kc % 4, :])
                        for m in range(16):
                            ps = pp.tile([128, T], F32, name=f"xwps{layer}{d}{m}",
                                         tag="xwps")
                            for kc in range(K):
                                nc.tensor.matmul(
                                    ps, wih_sb[kc][:, 128 * m:128 * (m + 1)], rhs[kc],
                                    start=(kc == 0), stop=(kc == K - 1))
                            nc.vector.tensor_copy(
                                xwt_sb[d][:, T * m:T * (m + 1)], ps)

            def scan_phase(layer):
                # gate layout (permuted on host): cols 0:4=i, 4:8=f, 8:12=o,
                # 12:16=g~ with g~ pre-activations DOUBLED (host scaled the
                # weights), so one sigmoid over all 16 cols gives
                # tanh(x) = 2*sigmoid(2x) - 1 for the g~ block.
                with tc.tile_pool(name=f"psg{layer}", bufs=4, space="PSUM") as p_g, \
                     tc.tile_pool(name=f"sg{layer}", bufs=3) as sgp:
                    for s in range(n_steps):
                        for d in (0, 1):
                            t = s if d == 0 else T - 1 - s
                            hh = hh_sb[layer][d]
                            if s == 0:
                                h_prev = h0_sb[layer][d]
                            else:
                                tp = t - 1 if d == 0 else t + 1
                                h_prev = hh[:, 4 * tp:4 * tp + 4]
                            g_all = p_g.tile([128, 16], F32, name=f"g{d}", tag=f"g{d}")
                            # seed PSUM with XW[t] via identity matmul (start),
                            # then accumulate the 64 matvec tiles on top
                            xwr = xwt_sb[d][:].rearrange("p (m t) -> p m t", t=T)
                            nc.tensor.matmul(g_all, ident_sb, xwr[:, :, t],
                                             start=True, stop=False,
                                             skip_group_check=True)
                            for m in range(16):
                                for kc in range(4):
                                    nc.tensor.matmul(
                                        g_all[:, m:m + 1],
                                        whh_sb[layer][d][kc][:, 128 * m:128 * (m + 1)],
                                        h_prev[:, kc:kc + 1],
                                        start=False,
                                        stop=(m == 15 and kc == 3),
                                        skip_group_check=True)
                            S = sgp.tile([128, 16], F32, name=f"S{d}", tag=f"S{d}")
                            nc.scalar.activation(S, g_all, AF.Sigmoid)
                            cc = c_sb[layer][d]
                            t1 = sgp.tile([128, 4], F32, name=f"t1{d}", tag=f"t1{d}")
                            u2 = sgp.tile([128, 4], F32, name=f"u2{d}", tag=f"u2{d}")
                            v3 = sgp.tile([128, 4], F32, name=f"v3{d}", tag=f"v3{d}")
                            nc.vector.tensor_mul(t1, S[:, 4:8], cc)
                            # u2 = 2*sigmoid(2g~) * i  (part of i*tanh(g~))
                            nc.vector.scalar_tensor_tensor(
                                u2, S[:, 12:16], 2.0, S[:, 0:4],
                                op0=ALU.mult, op1=ALU.mult)
                            nc.vector.tensor_add(v3, t1, u2)
                            nc.vector.tensor_sub(cc, v3, S[:, 0:4])
                            tct = sgp.tile([128, 4], F32, name=f"tc{d}", tag=f"tc{d}")
                            nc.scalar.activation(tct, cc, AF.Tanh)
                            nc.vector.tensor_mul(hh[:, 4 * t:4 * t + 4],
                                                 S[:, 8:12], tct)

            xwt_phase(0)
            # prefetch layer-1 Wih during the L0 scan (DMAs have no deps on
            # the scan, so the scheduler overlaps them with it)
            wih1_pre = [[], []]
            with tc.tile_pool(name="wihpre1", bufs=1) as wpre:
                for d in (0, 1):
                    for kc in range(8):
                        w = wpre.tile([128, G], BF, name=f"wihpre{d}{kc}")
                        nc.sync.dma_start(w, d_wihT1[d][128 * kc:128 * (kc + 1), :])
                        wih1_pre[d].append(w)
                scan_phase(0)
                # ones row for layer-1 bias trick: x1 dim 416 -> (c=3, p=32) of
                # fwd hist (DVE start partition must be 32-aligned; 416 is pad)
                hh0f_r = hh_sb[0][0][:].rearrange("p (t c) -> p c t", c=4)
                nc.vector.memset(hh0f_r[32:33, 3, :], 1.0)
                xwt_phase(1, preloaded=wih1_pre)
            scan_phase(1)
            hh1f_r = hh_sb[1][0][:].rearrange("p (t c) -> p c t", c=4)
            nc.vector.memset(hh1f_r[32:33, 3, :], 1.0)

            # ---------- pairwise scoring ----------
            def hvecT_chunk(kc, jt):
                """lhsT [128, 128]: hvec.T rows [128kc:128kc+128], cols [128jt:+128]."""
                hhr = hh_sb[1][kc // 4][:].rearrange("p (t c) -> p c t", c=4)
                return hhr[:, kc % 4, 128 * jt:128 * (jt + 1)]

            with tc.tile_pool(name="pw", bufs=1) as pw:
                pj_sb, pi_sb = [], []
                with tc.tile_pool(name="pwps", bufs=2, space="PSUM") as pwps:
                    for jt in range(2):
                        ps = pwps.tile([128, MLP], F32, name=f"pjps{jt}", tag="projps")
                        for kc in range(8):
                            nc.tensor.matmul(ps, hvecT_chunk(kc, jt), w1bT_sb[kc],
                                             start=(kc == 0), stop=(kc == 7))
                        pj = pw.tile([128, MLP], BF, name=f"pj{jt}")
                        nc.vector.tensor_copy(pj, ps)
                        pj_sb.append(pj)
                    for jt in range(2):
                        ps = pwps.tile([128, MLP], F32, name=f"pips{jt}", tag="projps")
                        for kc in range(8):
                            nc.tensor.matmul(ps, hvecT_chunk(kc, jt), w1aT_sb[kc],
                                             start=(kc == 0), stop=(kc == 7))
                        pi = pw.tile([128, MLP], BF, name=f"pi{jt}")
                        nc.vector.tensor_copy(pi, ps)
                        pi_sb.append(pi)
                    # select this core's 32 head rows: pik = skT.T @ pi  [32, 400]
                    ps = pwps.tile([RPC, MLP], F32, name="pikps", tag="projps")
                    for kc in range(2):
                        nc.tensor.matmul(ps, skT_sb[kc], pi_sb[kc],
                                         start=(kc == 0), stop=(kc == 1))
                    pik = pw.tile([RPC, MLP], BF, name="pik")
                    nc.vector.tensor_copy(pik, ps)
                pik_flat = pw.tile([1, RPC * MLP], BF, name="pikflat")
                nc.sync.dma_start(
                    pik_flat[:].rearrange("p (a b) -> p a b", a=RPC),
                    pik)
                scoresT = [pw.tile([128, RPC], F32, name=f"scoresT{jc}")
                           for jc in range(2)]

                with tc.tile_pool(name="bps", bufs=2, space="PSUM") as bps, \
                     tc.tile_pool(name="bsb", bufs=3) as bsb:
                    for ig in range(RPC // 4):
                        for jc in range(2):
                            # 512-strided slots: matmul out must stay in 1 bank
                            B_ps = bps.tile([128, 4 * 512], F32, name=f"bps{ig}{jc}",
                                            tag="bps")
                            for l in range(4):
                                r = 4 * ig + l
                                nc.tensor.matmul(
                                    B_ps[:, 512 * l:512 * l + MLP], ones_sb,
                                    pik_flat[:, MLP * r:MLP * (r + 1)],
                                    start=True, stop=True)
                            B_sb = bsb.tile([128, 4 * MLP], BF, name=f"bsb{ig}{jc}",
                                            tag="bsb")
                            for l in range(4):
                                nc.vector.tensor_add(
                                    B_sb[:, MLP * l:MLP * (l + 1)],
                                    B_ps[:, 512 * l:512 * l + MLP], pj_sb[jc])
                            Tact = bsb.tile([128, 4 * MLP], BF, name=f"tact{ig}{jc}",
                                            tag="tact")
                            nc.scalar.activation(Tact, B_sb, AF.Tanh)
                            scr = bsb.tile([128, 4 * MLP], F32, name=f"scr{ig}{jc}",
                                           tag="scr")
                            for l in range(4):
                                nc.vector.tensor_mul(
                                    scr[:, MLP * l:MLP * (l + 1)],
                                    Tact[:, MLP * l:MLP * (l + 1)], w2rep_sb)
                            nc.vector.tensor_reduce(
                                scoresT[jc][:, 4 * ig:4 * ig + 4].rearrange(
                                    "p (a b) -> p a b", b=1),
                                scr[:].rearrange("p (a b) -> p a b", a=4),
                                axis=mybir.AxisListType.X, op=ALU.add)
                for jc in range(2):
                    nc.sync.dma_start(d_out[128 * jc:128 * (jc + 1), :], scoresT[jc])

    nc.compile()
    _PROG_CACHE[key] = nc
    return nc


def _try_hw(n_steps, cores=1):
    """Debug helper: run a reduced-step program on HW with random inputs."""
    import jax
    from concourse import bass_utils
    from concourse.bass_interp import get_hw_module
    cpu = jax.devices("cpu")[0]
    rng = np.random.default_rng(0)
    fake = {
        "words": rng.integers(0, 50000, (T,)), "tags": rng.integers(0, 50, (T,)),
        "word_emb": rng.standard_normal((50000, 300), np.float32),
        "tag_emb": rng.standard_normal((50, 100), np.float32),
        "Wih0": rng.standard_normal((2, 1600, 400), np.float32) * 0.05,
        "Whh0": rng.standard_normal((2, 1600, 400), np.float32) * 0.05,
        "bih0": rng.standard_normal((2, 1600), np.float32) * 0.05,
        "bhh0": rng.standard_normal((2, 1600), np.float32) * 0.05,
        "Wih1": rng.standard_normal((2, 1600, 800), np.float32) * 0.05,
        "Whh1": rng.standard_normal((2, 1600, 400), np.float32) * 0.05,
        "bih1": rng.standard_normal((2, 1600), np.float32) * 0.05,
        "bhh1": rng.standard_normal((2, 1600), np.float32) * 0.05,
        "W1": rng.standard_normal((400, 1600), np.float32) * 0.05,
        "b1": rng.standard_normal((400,), np.float32) * 0.05,
        "W2": rng.standard_normal((1, 400), np.float32) * 0.05,
        "b2": rng.standard_normal((1,), np.float32) * 0.05,
        "h0": rng.standard_normal((2, 2, 400), np.float32),
        "c0": rng.standard_normal((2, 2, 400), np.float32),
    }
    nc = _get_program(n_steps)
    in_maps, I = _prep_inputs(fake)
    old = nc.m
    nc.m = get_hw_module(nc.m)
    try:
        res = bass_utils.run_bass_kernel_spmd(nc, in_maps[:cores],
                                              core_ids=list(range(cores)))
    finally:
        nc.m = old
    print(f"n_steps={n_steps} cores={cores}: OK,",
          res.results[0]["scores_t"].shape)


def _prep_inputs(inputs):
    """Host-side prep: gather embeddings, build padded/permuted device tensors."""
    I = {k: np.asarray(v) for k, v in inputs.items()}
    x = np.concatenate([I["word_emb"][I["words"]], I["tag_emb"][I["tags"]]],
                       axis=1).astype(np.float32)          # [T, 400]
    xT = np.zeros((HP, T), np.float32)
    xT[:H] = x.T
    xT[H] = 1.0                                            # bias row

    common = {"xT": xT.astype(BF16)}
    for l in (0, 1):
        Din = H if l == 0 else 2 * H
        DinP = HP if l == 0 else D1P
        for di, d in enumerate("fb"):
            wih = _permute_pad_gate_rows(I[f"Wih{l}"][di])  # [2048, Din]
            if l == 0:
                wihp = _pad_cols(wih, HP)                   # [2048, 512]
            else:
                wihp = np.zeros((G, D1P), np.float32)
                wihp[:, :H] = wih[:, :H]                    # fwd part
                wihp[:, HP:HP + H] = wih[:, H:2 * H]        # bwd part
            bias = _permute_pad_gate_vec(I[f"bih{l}"][di] + I[f"bhh{l}"][di])
            # bias column: layer 0's ones-row is xT row 400; layer 1's is the
            # hist pad position 416 (partition-32-aligned for the memset)
            wihp[:, H if l == 0 else 416] += bias
            wihp[1536:] *= 2.0      # g~ block doubled: tanh(x) = 2*sig(2x)-1
            common[f"wihT{l}{d}"] = np.ascontiguousarray(wihp.T).astype(BF16)

            whh = _pad_cols(_permute_pad_gate_rows(I[f"Whh{l}"][di]), HP)
            whh[1536:] *= 2.0
            common[f"whhT{l}{d}"] = np.ascontiguousarray(whh.T).astype(BF16)

            common[f"h0_{l}{d}"] = _h_tile(I["h0"][l, di]).astype(BF16)
            common[f"c0_{l}{d}"] = _h_tile(I["c0"][l, di]).astype(np.float32)

    W1 = I["W1"].astype(np.float32)                         # [400, 1600]
    W1a, W1b = W1[:, :2 * H], W1[:, 2 * H:]                 # [400, 800] each

    def mlp_T(W, bias=None):
        Wp = np.zeros((MLP, D1P), np.float32)
        Wp[:, :H] = W[:, :H]
        Wp[:, HP:HP + H] = W[:, H:]
        if bias is not None:
            Wp[:, 416] += bias                              # hvec ones-row at 416
        return np.ascontiguousarray(Wp.T).astype(BF16)      # [1024, 400]

    common["w1aT"] = mlp_T(W1a, I["b1"].astype(np.float32))
    common["w1bT"] = mlp_T(W1b)
    common["w2rep"] = np.broadcast_to(I["W2"][0].astype(np.float32),
                                      (128, MLP)).astype(BF16).copy()
    common["ident"] = np.eye(128, dtype=np.float32).astype(BF16)

    in_maps = []
    for k in range(NCORES):
        m = dict(common)
        sk = np.zeros((T, RPC), np.float32)
        sk[RPC * k + np.arange(RPC), np.arange(RPC)] = 1.0
        m["skT"] = sk.astype(BF16)
        in_maps.append(m)
    return in_maps, I


def _ensure_ntff_hook():
    """Shim antenv.axon_hooks (absent in this image) so trace=True works."""
    import sys
    import types
    import antenv
    if hasattr(antenv, "axon_hooks") or "antenv.axon_hooks" in sys.modules:
        return
    hook = None
    try:
        from trn_agent_boot.trn_boot import _ntff_profile_via_ctypes
        hook = _ntff_profile_via_ctypes("/opt/axon/libaxon_pjrt.so")
    except Exception:
        hook = None
    mod = types.ModuleType("antenv.axon_hooks")
    state = {"hook": hook}
    mod.get_axon_ntff_profile_hook = lambda: state["hook"]
    mod.set_axon_ntff_profile_hook = lambda h: state.update(hook=h)
    sys.modules["antenv.axon_hooks"] = mod
    antenv.axon_hooks = mod


def kernel(**inputs):
    from concourse import bass_utils
    from concourse.bass_interp import get_hw_module

    nc = _get_program()
    in_maps, I = _prep_inputs(inputs)

    trace = bool(int(os.environ.get("KERNEL_TRACE", "0")))
    if trace:
        _ensure_ntff_hook()
    old_m = nc.m
    nc.m = get_hw_module(nc.m)
    try:
        res = bass_utils.run_bass_kernel_spmd(
            nc, in_maps, core_ids=list(range(NCORES)), trace=trace)
    finally:
        nc.m = old_m
    if trace and res.exec_time_ns is not None:
        print(f"HW exec time: {res.exec_time_ns} ns")
        kernel.last_exec_time_ns = res.exec_time_ns

    scores = np.zeros((T, T), np.float32)
    for k in range(NCORES):
        scores[RPC * k:RPC * (k + 1), :] = res.results[k]["scores_t"].T
    scores += float(I["b2"][0])
    scores[np.arange(T), np.arange(T)] = 0.0
    return scores

